# revision 21
# baseline (speedup 1.0000x reference)
"""GCN NodeAttributeAggregator on 8 Trainium2 NeuronCores.

Strategy (node-sharded, dst-partitioned edges):
  - Host precomputes index metadata: degrees (with self-loops), dinv=rsqrt(deg),
    per-core edge lists partitioned by dst owner, grouped by (dst-block of 128,
    src<32768 vs src>=32768 for int16 gather indices), padded to 128-edge tiles.
  - Device per core: dma_gather rows of a pre-scaled node table (xs = x*dinv),
    scatter-by-matmul: one-hot P matrices (built on DVE via iota + is_equal)
    contract each 128-edge tile into a 128-dst PSUM accumulator.
  - Dense 256x256 matmuls run in float32r (full PE rate) feature-major, with
    PE transposes at layout boundaries.
  - Algebra: GCN layer out = D^-1/2 (A+I) D^-1/2 h W.  Row scaling commutes
    with right matmuls, relu commutes with positive row scaling, and
    agg(h W) = agg(h) W, so:
      L1 (fused W_pre@W1): u' = (scatter(xs) + xs_dst) * dinv;
          g1 = relu(u' @ (W_pre W1) + b1 + rank1(b_pre)) * dinv
      L2: v' = (scatter(g1) + g1_dst) * dinv;
          y  = relu(v' @ W2 + b2) @ W_post + b_post
  - Two SPMD launches; host concatenates g1 slices between them.
"""

import dataclasses
import ml_dtypes
import numpy as np

import concourse.bacc as bacc
import concourse.bass as bass
import concourse.tile as tile
import concourse.mybir as mybir
from concourse.bass_utils import run_bass_kernel_spmd
from concourse.masks import make_identity

P = 128
SCAT_BF16 = True  # bf16 gather tables + P matrices (PE 1 cyc/row + FWL)
NSWQ = 4  # SWDGE queues
GCH = 4  # gather tiles per dma_gather call (ring holds 2 calls)
f32 = mybir.dt.float32
f32r = mybir.dt.float32r
bf16 = mybir.dt.bfloat16
gdt = bf16 if SCAT_BF16 else f32r
i16 = mybir.dt.int16
i32 = mybir.dt.int32


@dataclasses.dataclass
class Cfg:
    n_nodes: int = 50000
    d: int = 256
    nc: int = 8
    split: int = 32768
    dense_n: int = 512

    @property
    def nloc(self):
        return self.n_nodes // self.nc

    @property
    def nblk(self):
        return (self.nloc + P - 1) // P

    @property
    def npad(self):
        return self.nblk * P

    @property
    def nhi(self):
        return self.n_nodes - self.split


# ---------------------------------------------------------------- host prep


def _wrap16(vals):
    """[n] -> [128, n//16] int16: value i at [i%16, i//16], replicated x8."""
    w = vals.reshape(-1, 16).T
    return np.tile(w, (8, 1)).astype(np.int16)


def _prep_edges(cfg, src, dst):
    """Partition edges by dst owner; group by (block, src-range); pad.

    Returns (TA, TB, per-core dict of idx/slot planes) with identical
    compile-time schedule (TA, TB) across cores.
    """
    nl, nb = cfg.nloc, cfg.nblk
    owner = dst // nl
    loc = dst - owner * nl
    blk = loc // P
    slot = loc - blk * P
    grp = (src >= cfg.split).astype(np.int64)

    key = (owner * nb + blk) * 2 + grp
    nkeys = cfg.nc * nb * 2
    counts = np.bincount(key, minlength=nkeys)
    T_cbg = -(-counts // P).reshape(cfg.nc, nb, 2)  # ceil
    T = T_cbg.max(axis=0)  # [nb, 2] tiles per (block, grp), same for all cores
    TA, TB = T[:, 0].copy(), T[:, 1].copy()
    padlen = T * P  # [nb, 2] padded edge rows

    # per-core stream layout: for b: [A rows pad][B rows pad]
    seg_len = padlen.sum(axis=1)  # per block
    blk_base = np.concatenate([[0], np.cumsum(seg_len)])[:-1]  # [nb]
    rows_total = int(seg_len.sum())
    # row base for (b, g)
    gbase = np.stack([blk_base, blk_base + padlen[:, 0]], axis=1)  # [nb, 2]

    # stable order by key, then rank within group
    order = np.argsort(key, kind="stable")
    skey = key[order]
    group_start = np.concatenate([[0], np.cumsum(np.bincount(skey, minlength=nkeys))])
    rank = np.arange(len(src)) - group_start[skey]

    rows = gbase[blk[order], grp[order]] + rank
    cores = owner[order]
    idxv = (src[order] - grp[order] * cfg.split).astype(np.int16)
    slotv = slot[order].astype(np.float32)

    idx_flat = np.zeros((cfg.nc, rows_total), np.int16)
    slot_flat = np.full((cfg.nc, rows_total), 300.0, np.float32)
    idx_flat[cores, rows] = idxv
    slot_flat[cores, rows] = slotv

    per_core = []
    for c in range(cfg.nc):
        idxp = _wrap16(idx_flat[c])  # [128, rows_total//16]
        slotp = slot_flat[c].reshape(-1, P).T.copy()  # [128, ntiles]
        per_core.append({"idxp": idxp, "slotp": slotp})
    return TA, TB, per_core


def _wrap_cols(vec, nblk, npad):
    """[npad] -> [128, nblk] with [p, b] = vec[b*128+p]."""
    v = np.zeros(npad, np.float32)
    v[: len(vec)] = vec
    return v.reshape(nblk, P).T.copy()


# ------------------------------------------------------------- device build


def build_launch(cfg, mode, TA, TB, has_bpre=False):
    """mode 1: out = relu(u' @ WA + b1 [+ rank1]) * dinv   (writes g1)
    mode 2: out = relu(v' @ W2 + b2) @ W_post + b_post     (writes y)
    """
    nb, npad, d = cfg.nblk, cfg.npad, cfg.d
    ntiles = int((TA + TB).sum())
    nidxcol = ntiles * 8
    tmaxA = max(int(TA.max()), 1)
    tmaxB = max(int(TB.max()), 1)
    tmaxP = max(int((TA + TB).max()), 1)

    nc = bacc.Bacc("TRN2", target_bir_lowering=False, debug=False,
                   num_devices=cfg.nc, num_swdge_queues=NSWQ)

    tablo = nc.dram_tensor("tablo", [cfg.split, d], gdt, kind="ExternalInput")
    tabhi = nc.dram_tensor("tabhi", [cfg.nhi, d], gdt, kind="ExternalInput")
    idxp_d = nc.dram_tensor("idxp", [P, nidxcol], i16, kind="ExternalInput")
    slotp_d = nc.dram_tensor("slotp", [P, ntiles], f32, kind="ExternalInput")
    dinvw_d = nc.dram_tensor("dinvw", [P, nb], f32, kind="ExternalInput")
    nw = 1 if mode == 1 else 2
    w_d = [nc.dram_tensor(f"w{i}", [d, d], gdt, kind="ExternalInput")
           for i in range(nw)]
    bias_d = [nc.dram_tensor(f"bias{i}", [P, d // P], f32, kind="ExternalInput")
              for i in range(nw)]
    if has_bpre:
        c1rep_d = nc.dram_tensor("c1rep", [P, npad], f32, kind="ExternalInput")
        v1w_d = nc.dram_tensor("v1w", [P, d // P], f32, kind="ExternalInput")
    out_d = nc.dram_tensor("out", [npad, d], f32, kind="ExternalOutput")

    kd = d // P  # feature k-tiles (2)

    with tile.TileContext(nc) as tc:
        with (
            tc.tile_pool(name="const", bufs=1) as cpool,
            tc.tile_pool(name="gA", bufs=6) as gApool,
            tc.tile_pool(name="gB", bufs=6) as gBpool,
            tc.tile_pool(name="pmat", bufs=2) as ppool,
            tc.tile_pool(name="work", bufs=3) as wpool,
            tc.tile_pool(name="stage", bufs=3) as stpool,
            tc.tile_pool(name="zslab", bufs=2) as zpool,
            tc.tile_pool(name="apsum", bufs=4, space="PSUM") as apsum,
            tc.tile_pool(name="trpsum", bufs=2, space="PSUM") as trpsum,
            tc.tile_pool(name="dpsum", bufs=2, space="PSUM") as dpsum,
        ):
            # ---- constants
            iota_i = cpool.tile([P, P], i32)
            nc.gpsimd.iota(iota_i[:], pattern=[[1, P]], base=0,
                           channel_multiplier=0)
            iota_f = cpool.tile([P, P], f32)
            nc.vector.tensor_copy(iota_f[:], iota_i[:])
            ident = cpool.tile([P, P], f32)
            make_identity(nc, ident[:])
            idxp_t = cpool.tile([P, nidxcol], i16)
            nc.sync.dma_start(idxp_t[:], idxp_d[:])
            slotp_t = cpool.tile([P, ntiles], f32)
            nc.sync.dma_start(slotp_t[:], slotp_d[:])
            dinvw_t = cpool.tile([P, nb], f32)
            nc.sync.dma_start(dinvw_t[:], dinvw_d[:])
            w_t = []  # [stage][k][m] -> [128,128] f32r lhsT tiles
            for i in range(nw):
                tiles = []
                for k in range(kd):
                    row = []
                    for m in range(kd):
                        wt = cpool.tile([P, P], gdt, name=f"wt{i}_{k}_{m}",
                                        tag=f"wt{i}_{k}_{m}")
                        nc.sync.dma_start(
                            wt[:], w_d[i][k * P:(k + 1) * P, m * P:(m + 1) * P])
                        row.append(wt)
                    tiles.append(row)
                w_t.append(tiles)
            bias_t = []
            for i in range(nw):
                bt = cpool.tile([P, kd], f32, name=f"bt{i}", tag=f"bt{i}")
                nc.sync.dma_start(bt[:], bias_d[i][:])
                bias_t.append(bt)
            if has_bpre:
                c1rep_t = cpool.tile([P, npad], f32)
                nc.sync.dma_start(c1rep_t[:], c1rep_d[:])
                v1w_t = cpool.tile([P, kd], f32)
                nc.sync.dma_start(v1w_t[:], v1w_d[:])

            # feature-major activations, one tile per dense node-slice
            nsl = (npad + cfg.dense_n - 1) // cfg.dense_n
            uT_s = [cpool.tile([P, kd, min(cfg.dense_n, npad - i * cfg.dense_n)],
                               gdt, name=f"uTs{i}", tag=f"uTs{i}")
                    for i in range(nsl)]

            # ---- aggregation pass
            gq = [0]
            icol = 0  # idx plane column cursor (units of 8 per tile)
            tcol = 0  # slot plane column cursor (1 per tile)
            for b in range(nb):
                ta, tb = int(TA[b]), int(TB[b])
                tbt = ta + tb
                assert tbt > 0
                psum_a = apsum.tile([P, d], f32, space="PSUM", tag="psum_a")
                if tbt:
                    p_all = ppool.tile([P, tmaxP, P], gdt, tag="pmat")
                    nc.vector.tensor_tensor(
                        out=p_all[:, 0:tbt, :],
                        in0=slotp_t[:, tcol:tcol + tbt, None].to_broadcast(
                            [P, tbt, P]),
                        in1=iota_f[:, None, :].to_broadcast([P, tbt, P]),
                        op=mybir.AluOpType.is_equal)
                j = 0
                for pool_g, tab_ap, tcnt, gtag in (
                    (gApool, tablo, ta, "gA"),
                    (gBpool, tabhi, tb, "gB"),
                ):
                    for c0 in range(0, tcnt, GCH):
                        cn = min(GCH, tcnt - c0)
                        gt = pool_g.tile([P, GCH, d], gdt, tag=gtag,
                                         name=f"g_{b}_{gtag}_{c0}")
                        nc.gpsimd.dma_gather(
                            out_ap=gt[:, 0:cn, :], in_ap=tab_ap[:],
                            idxs_ap=idxp_t[:, icol:icol + cn * 8],
                            num_idxs=cn * P, num_idxs_reg=cn * P, elem_size=d,
                            queue_num=gq[0] % NSWQ)
                        gq[0] += 1
                        icol += cn * 8
                        for t in range(cn):
                            nc.tensor.matmul(
                                psum_a[:], lhsT=p_all[:, j, :], rhs=gt[:, t, :],
                                start=(j == 0), stop=(j == tbt - 1))
                            j += 1
                tcol += tbt

                # epilogue: u' = psum * dinv (self-loops are gathered edges)
                u2 = wpool.tile([P, d], f32, tag="u2")
                nc.scalar.mul(u2[:], psum_a[:], dinvw_t[:, b:b + 1])
                for m in range(kd):
                    ptr = trpsum.tile([P, P], f32, space="PSUM", tag="ptr")
                    nc.tensor.transpose(out=ptr[:], in_=u2[:, m * P:(m + 1) * P],
                                        identity=ident[:])
                    sl, off = divmod(b * P, cfg.dense_n)
                    nc.vector.tensor_copy(uT_s[sl][:, m, off:off + P], ptr[:])

            # ---- dense pass over node slices
            for s0 in range(0, npad, cfg.dense_n):
                ns = min(cfg.dense_n, npad - s0)
                pz = [dpsum.tile([P, ns], f32, space="PSUM", tag="dps",
                                 name=f"pz{dt}") for dt in range(kd)]
                for dt in range(kd):
                    for m in range(kd):
                        nc.tensor.matmul(
                            pz[dt][:], lhsT=w_t[0][m][dt][:],
                            rhs=uT_s[s0 // cfg.dense_n][:, m, 0:ns],
                            start=(m == 0), stop=(m == kd - 1))
                if has_bpre:
                    for dt in range(kd):
                        tmp = wpool.tile([P, cfg.dense_n], f32, tag="r1")
                        nc.vector.tensor_scalar_mul(
                            tmp[:, 0:ns], c1rep_t[:, s0:s0 + ns],
                            v1w_t[:, dt:dt + 1])
                        nc.vector.tensor_tensor(
                            out=pz[dt][:], in0=pz[dt][:], in1=tmp[:, 0:ns],
                            op=mybir.AluOpType.add)

                if mode == 1:
                    zr = zpool.tile([P, kd, cfg.dense_n], f32, tag="zr")
                    for dt in range(kd):
                        nc.scalar.activation(
                            zr[:, dt, 0:ns], pz[dt][:],
                            mybir.ActivationFunctionType.Relu,
                            bias=bias_t[0][:, dt:dt + 1], scale=1.0)
                    final = zr
                else:
                    rT = zpool.tile([P, kd, cfg.dense_n], gdt, tag="zr")
                    for dt in range(kd):
                        nc.scalar.activation(
                            rT[:, dt, 0:ns], pz[dt][:],
                            mybir.ActivationFunctionType.Relu,
                            bias=bias_t[0][:, dt:dt + 1], scale=1.0)
                    py = [dpsum.tile([P, ns], f32, space="PSUM", tag="dps",
                                     name=f"py{dt}") for dt in range(kd)]
                    for dt in range(kd):
                        for m in range(kd):
                            nc.tensor.matmul(
                                py[dt][:], lhsT=w_t[1][m][dt][:],
                                rhs=rT[:, m, 0:ns],
                                start=(m == 0), stop=(m == kd - 1))
                    yT = zpool.tile([P, kd, cfg.dense_n], f32, tag="yT")
                    for dt in range(kd):
                        nc.scalar.activation(
                            yT[:, dt, 0:ns], py[dt][:],
                            mybir.ActivationFunctionType.Identity,
                            bias=bias_t[1][:, dt:dt + 1], scale=1.0)
                    final = yT

                for jj in range(ns // P):
                    blk = (s0 + jj * P) // P
                    ost = stpool.tile([P, d], f32, tag="ost")
                    for dt in range(kd):
                        ptr2 = trpsum.tile([P, P], f32, space="PSUM", tag="ptr")
                        nc.tensor.transpose(
                            out=ptr2[:], in_=final[:, dt, jj * P:(jj + 1) * P],
                            identity=ident[:])
                        nc.vector.tensor_copy(
                            ost[:, dt * P:(dt + 1) * P], ptr2[:])
                    nc.sync.dma_start(out_d[blk * P:(blk + 1) * P, :], ost[:])

    nc.compile()
    return nc


# ------------------------------------------------------------------ driver


def _run(cfg, nc_prog, per_core_common, per_core_vars, trace=False):
    in_maps = []
    for c in range(cfg.nc):
        m = dict(per_core_common)
        m.update(per_core_vars[c])
        in_maps.append(m)
    res = run_bass_kernel_spmd(nc_prog, in_maps, core_ids=list(range(cfg.nc)),
                               trace=trace)
    return res


def gcn_forward(cfg, x, edge_index, W_pre, b_pre, W1, b1, W2, b2, W_post,
                b_post, trace=False, ret_times=None):
    x = np.asarray(x, np.float32)
    src = np.asarray(edge_index[0], np.int64)
    dst = np.asarray(edge_index[1], np.int64)
    W_pre, W1, W2, W_post = (np.asarray(w, np.float32)
                             for w in (W_pre, W1, W2, W_post))
    b_pre, b1, b2, b_post = (np.asarray(b, np.float32)
                             for b in (b_pre, b1, b2, b_post))

    n, d, nl, nb, npad = cfg.n_nodes, cfg.d, cfg.nloc, cfg.nblk, cfg.npad
    deg = (np.bincount(dst, minlength=n) + 1).astype(np.float64)
    dinv = (1.0 / np.sqrt(deg)).astype(np.float32)

    loops = np.arange(n, dtype=np.int64)
    src_all = np.concatenate([src, loops])
    dst_all = np.concatenate([dst, loops])
    TA, TB, edge_planes = _prep_edges(cfg, src_all, dst_all)

    xs = x * dinv[:, None]
    WA = (W_pre.astype(np.float64) @ W1.astype(np.float64)).astype(np.float32)

    has_bpre = bool(np.any(b_pre != 0))
    dinv_cols = [
        _wrap_cols(dinv[c * nl:(c + 1) * nl], nb, npad) for c in range(cfg.nc)]

    # ---------- launch 1
    prog1 = build_launch(cfg, 1, TA, TB, has_bpre=has_bpre)
    tdt = ml_dtypes.bfloat16 if SCAT_BF16 else np.float32
    common1 = {
        "tablo": xs[: cfg.split].astype(tdt),
        "tabhi": xs[cfg.split:].astype(tdt),
        "w0": WA.astype(tdt),
        "bias0": b1.reshape(d // P, P).T.copy(),
    }
    if has_bpre:
        v1 = (b_pre.astype(np.float64) @ W1.astype(np.float64)).astype(
            np.float32)
        common1["v1w"] = v1.reshape(d // P, P).T.copy()
        # c1[dst] = (s[dst] + dinv[dst]) * dinv[dst],  s = sum_e dinv[src]
        s = np.zeros(n, np.float64)
        np.add.at(s, dst, dinv[src].astype(np.float64))
        c1_full = ((s + dinv) * dinv).astype(np.float32)
    vars1 = []
    for c in range(cfg.nc):
        v = {
            "idxp": edge_planes[c]["idxp"],
            "slotp": edge_planes[c]["slotp"],
            "dinvw": dinv_cols[c],
        }
        if has_bpre:
            cl = np.zeros(npad, np.float32)
            cl[:nl] = c1_full[c * nl:(c + 1) * nl]
            v["c1rep"] = np.tile(cl, (P, 1))
        vars1.append(v)
    res1 = _run(cfg, prog1, common1, vars1, trace=trace)
    g1 = np.concatenate([res1.results[c]["out"][:nl] for c in range(cfg.nc)])
    g1 *= dinv[:, None]
    if ret_times is not None:
        ret_times.append(res1.exec_time_ns)

    # ---------- launch 2
    prog2 = build_launch(cfg, 2, TA, TB, has_bpre=False)
    common2 = {
        "tablo": g1[: cfg.split].astype(tdt),
        "tabhi": g1[cfg.split:].astype(tdt),
        "w0": W2.astype(tdt),
        "w1": W_post.astype(tdt),
        "bias0": b2.reshape(d // P, P).T.copy(),
        "bias1": b_post.reshape(d // P, P).T.copy(),
    }
    vars2 = []
    for c in range(cfg.nc):
        vars2.append({
            "idxp": edge_planes[c]["idxp"],
            "slotp": edge_planes[c]["slotp"],
            "dinvw": dinv_cols[c],
        })
    res2 = _run(cfg, prog2, common2, vars2, trace=trace)
    y = np.concatenate([res2.results[c]["out"][:nl] for c in range(cfg.nc)])
    if ret_times is not None:
        ret_times.append(res2.exec_time_ns)
    return y


def kernel(x, edge_index, W_pre, b_pre, W1, b1, W2, b2, W_post, b_post):
    cfg = Cfg()
    return gcn_forward(cfg, x, edge_index, W_pre, b_pre, W1, b1, W2, b2,
                       W_post, b_post)


# revision 22
# speedup vs baseline: 1.0882x; 1.0882x over previous
"""GCN NodeAttributeAggregator on 8 Trainium2 NeuronCores.

Strategy (node-sharded, dst-partitioned edges):
  - Host precomputes index metadata: degrees (with self-loops), dinv=rsqrt(deg),
    per-core edge lists partitioned by dst owner, grouped by (dst-block of 128,
    src<32768 vs src>=32768 for int16 gather indices), padded to 128-edge tiles.
  - Device per core: dma_gather rows of a pre-scaled node table (xs = x*dinv),
    scatter-by-matmul: one-hot P matrices (built on DVE via iota + is_equal)
    contract each 128-edge tile into a 128-dst PSUM accumulator.
  - Dense 256x256 matmuls run in float32r (full PE rate) feature-major, with
    PE transposes at layout boundaries.
  - Algebra: GCN layer out = D^-1/2 (A+I) D^-1/2 h W.  Row scaling commutes
    with right matmuls, relu commutes with positive row scaling, and
    agg(h W) = agg(h) W, so:
      L1 (fused W_pre@W1): u' = (scatter(xs) + xs_dst) * dinv;
          g1 = relu(u' @ (W_pre W1) + b1 + rank1(b_pre)) * dinv
      L2: v' = (scatter(g1) + g1_dst) * dinv;
          y  = relu(v' @ W2 + b2) @ W_post + b_post
  - Two SPMD launches; host concatenates g1 slices between them.
"""

import dataclasses
import ml_dtypes
import numpy as np

import concourse.bacc as bacc
import concourse.bass as bass
import concourse.tile as tile
import concourse.mybir as mybir
from concourse.bass_utils import run_bass_kernel_spmd
from concourse.masks import make_identity

P = 128
SCAT_BF16 = True  # bf16 gather tables + P matrices (PE 1 cyc/row + FWL)
NSWQ = 4  # SWDGE queues
GCH = 8  # gather tiles per dma_gather call
f32 = mybir.dt.float32
f32r = mybir.dt.float32r
bf16 = mybir.dt.bfloat16
gdt = bf16 if SCAT_BF16 else f32r
i16 = mybir.dt.int16
i32 = mybir.dt.int32


@dataclasses.dataclass
class Cfg:
    n_nodes: int = 50000
    d: int = 256
    nc: int = 8
    split: int = 32768
    dense_n: int = 512

    @property
    def nloc(self):
        return self.n_nodes // self.nc

    @property
    def nblk(self):
        return (self.nloc + P - 1) // P

    @property
    def npad(self):
        return self.nblk * P

    @property
    def nhi(self):
        return self.n_nodes - self.split


# ---------------------------------------------------------------- host prep


def _wrap16(vals):
    """[n] -> [128, n//16] int16: value i at [i%16, i//16], replicated x8."""
    w = vals.reshape(-1, 16).T
    return np.tile(w, (8, 1)).astype(np.int16)


def _prep_edges(cfg, src, dst):
    """Partition edges by dst owner; group by (block, src-range); pad.

    Returns (TA, TB, per-core dict of idx/slot planes) with identical
    compile-time schedule (TA, TB) across cores.
    """
    nl, nb = cfg.nloc, cfg.nblk
    owner = dst // nl
    loc = dst - owner * nl
    blk = loc // P
    slot = loc - blk * P
    grp = (src >= cfg.split).astype(np.int64)

    key = (owner * nb + blk) * 2 + grp
    nkeys = cfg.nc * nb * 2
    counts = np.bincount(key, minlength=nkeys)
    T_cbg = -(-counts // P).reshape(cfg.nc, nb, 2)  # ceil
    T = T_cbg.max(axis=0)  # [nb, 2] tiles per (block, grp), same for all cores
    TA, TB = T[:, 0].copy(), T[:, 1].copy()
    padlen = T * P  # [nb, 2] padded edge rows

    # per-core stream layout: for b: [A rows pad][B rows pad]
    seg_len = padlen.sum(axis=1)  # per block
    blk_base = np.concatenate([[0], np.cumsum(seg_len)])[:-1]  # [nb]
    rows_total = int(seg_len.sum())
    # row base for (b, g)
    gbase = np.stack([blk_base, blk_base + padlen[:, 0]], axis=1)  # [nb, 2]

    # stable order by key, then rank within group
    order = np.argsort(key, kind="stable")
    skey = key[order]
    group_start = np.concatenate([[0], np.cumsum(np.bincount(skey, minlength=nkeys))])
    rank = np.arange(len(src)) - group_start[skey]

    rows = gbase[blk[order], grp[order]] + rank
    cores = owner[order]
    idxv = (src[order] - grp[order] * cfg.split).astype(np.int16)
    slotv = slot[order].astype(np.float32)

    idx_flat = np.zeros((cfg.nc, rows_total), np.int16)
    slot_flat = np.full((cfg.nc, rows_total), 300.0, np.float32)
    idx_flat[cores, rows] = idxv
    slot_flat[cores, rows] = slotv

    per_core = []
    for c in range(cfg.nc):
        idxp = _wrap16(idx_flat[c])  # [128, rows_total//16]
        slotp = slot_flat[c].reshape(-1, P).T.copy()  # [128, ntiles]
        per_core.append({"idxp": idxp, "slotp": slotp})
    return TA, TB, per_core


def _wrap_cols(vec, nblk, npad):
    """[npad] -> [128, nblk] with [p, b] = vec[b*128+p]."""
    v = np.zeros(npad, np.float32)
    v[: len(vec)] = vec
    return v.reshape(nblk, P).T.copy()


# ------------------------------------------------------------- device build


def build_launch(cfg, mode, TA, TB, has_bpre=False):
    """mode 1: out = relu(u' @ WA + b1 [+ rank1]) * dinv   (writes g1)
    mode 2: out = relu(v' @ W2 + b2) @ W_post + b_post     (writes y)
    """
    nb, npad, d = cfg.nblk, cfg.npad, cfg.d
    ntiles = int((TA + TB).sum())
    nidxcol = ntiles * 8
    tmaxA = max(int(TA.max()), 1)
    tmaxB = max(int(TB.max()), 1)
    tmaxP = max(int((TA + TB).max()), 1)

    nc = bacc.Bacc("TRN2", target_bir_lowering=False, debug=False,
                   num_devices=cfg.nc, num_swdge_queues=NSWQ)

    tablo = nc.dram_tensor("tablo", [cfg.split, d], gdt, kind="ExternalInput")
    tabhi = nc.dram_tensor("tabhi", [cfg.nhi, d], gdt, kind="ExternalInput")
    idxp_d = nc.dram_tensor("idxp", [P, nidxcol], i16, kind="ExternalInput")
    slotp_d = nc.dram_tensor("slotp", [P, ntiles], f32, kind="ExternalInput")
    dinvw_d = nc.dram_tensor("dinvw", [P, nb], f32, kind="ExternalInput")
    nw = 1 if mode == 1 else 2
    w_d = [nc.dram_tensor(f"w{i}", [d, d], gdt, kind="ExternalInput")
           for i in range(nw)]
    bias_d = [nc.dram_tensor(f"bias{i}", [P, d // P], f32, kind="ExternalInput")
              for i in range(nw)]
    if has_bpre:
        c1rep_d = nc.dram_tensor("c1rep", [P, npad], f32, kind="ExternalInput")
        v1w_d = nc.dram_tensor("v1w", [P, d // P], f32, kind="ExternalInput")
    out_d = nc.dram_tensor("out", [npad, d], f32, kind="ExternalOutput")

    kd = d // P  # feature k-tiles (2)

    with tile.TileContext(nc) as tc:
        with (
            tc.tile_pool(name="const", bufs=1) as cpool,
            tc.tile_pool(name="gA", bufs=6) as gApool,
            tc.tile_pool(name="gB", bufs=6) as gBpool,
            tc.tile_pool(name="pmat", bufs=2) as ppool,
            tc.tile_pool(name="work", bufs=3) as wpool,
            tc.tile_pool(name="stage", bufs=3) as stpool,
            tc.tile_pool(name="zslab", bufs=2) as zpool,
            tc.tile_pool(name="apsum", bufs=4, space="PSUM") as apsum,
            tc.tile_pool(name="trpsum", bufs=2, space="PSUM") as trpsum,
            tc.tile_pool(name="dpsum", bufs=2, space="PSUM") as dpsum,
        ):
            # ---- constants
            iota_i = cpool.tile([P, P], i32)
            nc.gpsimd.iota(iota_i[:], pattern=[[1, P]], base=0,
                           channel_multiplier=0)
            iota_f = cpool.tile([P, P], f32)
            nc.vector.tensor_copy(iota_f[:], iota_i[:])
            ident = cpool.tile([P, P], f32)
            make_identity(nc, ident[:])
            idxp_t = cpool.tile([P, nidxcol], i16)
            nc.sync.dma_start(idxp_t[:], idxp_d[:])
            slotp_t = cpool.tile([P, ntiles], f32)
            nc.sync.dma_start(slotp_t[:], slotp_d[:])
            dinvw_t = cpool.tile([P, nb], f32)
            nc.sync.dma_start(dinvw_t[:], dinvw_d[:])
            w_t = []  # [stage][k][m] -> [128,128] f32r lhsT tiles
            for i in range(nw):
                tiles = []
                for k in range(kd):
                    row = []
                    for m in range(kd):
                        wt = cpool.tile([P, P], gdt, name=f"wt{i}_{k}_{m}",
                                        tag=f"wt{i}_{k}_{m}")
                        nc.sync.dma_start(
                            wt[:], w_d[i][k * P:(k + 1) * P, m * P:(m + 1) * P])
                        row.append(wt)
                    tiles.append(row)
                w_t.append(tiles)
            bias_t = []
            for i in range(nw):
                bt = cpool.tile([P, kd], f32, name=f"bt{i}", tag=f"bt{i}")
                nc.sync.dma_start(bt[:], bias_d[i][:])
                bias_t.append(bt)
            if has_bpre:
                c1rep_t = cpool.tile([P, npad], f32)
                nc.sync.dma_start(c1rep_t[:], c1rep_d[:])
                v1w_t = cpool.tile([P, kd], f32)
                nc.sync.dma_start(v1w_t[:], v1w_d[:])

            # feature-major activations, one tile per dense node-slice
            nsl = (npad + cfg.dense_n - 1) // cfg.dense_n
            uT_s = [cpool.tile([P, kd, min(cfg.dense_n, npad - i * cfg.dense_n)],
                               gdt, name=f"uTs{i}", tag=f"uTs{i}")
                    for i in range(nsl)]

            # ---- aggregation pass
            gq = [0]
            icol = 0  # idx plane column cursor (units of 8 per tile)
            tcol = 0  # slot plane column cursor (1 per tile)
            for b in range(nb):
                ta, tb = int(TA[b]), int(TB[b])
                tbt = ta + tb
                assert tbt > 0
                psum_a = apsum.tile([P, d], f32, space="PSUM", tag="psum_a")
                if tbt:
                    p_all = ppool.tile([P, tmaxP, P], gdt, tag="pmat")
                    nc.vector.tensor_tensor(
                        out=p_all[:, 0:tbt, :],
                        in0=slotp_t[:, tcol:tcol + tbt, None].to_broadcast(
                            [P, tbt, P]),
                        in1=iota_f[:, None, :].to_broadcast([P, tbt, P]),
                        op=mybir.AluOpType.is_equal)
                j = 0
                for pool_g, tab_ap, tcnt, gtag in (
                    (gApool, tablo, ta, "gA"),
                    (gBpool, tabhi, tb, "gB"),
                ):
                    for c0 in range(0, tcnt, GCH):
                        cn = min(GCH, tcnt - c0)
                        gt = pool_g.tile([P, GCH, d], gdt, tag=gtag,
                                         name=f"g_{b}_{gtag}_{c0}")
                        nc.gpsimd.dma_gather(
                            out_ap=gt[:, 0:cn, :], in_ap=tab_ap[:],
                            idxs_ap=idxp_t[:, icol:icol + cn * 8],
                            num_idxs=cn * P, num_idxs_reg=cn * P, elem_size=d,
                            queue_num=gq[0] % NSWQ)
                        gq[0] += 1
                        icol += cn * 8
                        for t in range(cn):
                            nc.tensor.matmul(
                                psum_a[:], lhsT=p_all[:, j, :], rhs=gt[:, t, :],
                                start=(j == 0), stop=(j == tbt - 1))
                            j += 1
                tcol += tbt

                # epilogue: u' = psum * dinv (self-loops are gathered edges)
                u2 = wpool.tile([P, d], f32, tag="u2")
                nc.scalar.mul(u2[:], psum_a[:], dinvw_t[:, b:b + 1])
                for m in range(kd):
                    ptr = trpsum.tile([P, P], f32, space="PSUM", tag="ptr")
                    nc.tensor.transpose(out=ptr[:], in_=u2[:, m * P:(m + 1) * P],
                                        identity=ident[:])
                    sl, off = divmod(b * P, cfg.dense_n)
                    nc.vector.tensor_copy(uT_s[sl][:, m, off:off + P], ptr[:])

            # ---- dense pass over node slices
            for s0 in range(0, npad, cfg.dense_n):
                ns = min(cfg.dense_n, npad - s0)
                pz = [dpsum.tile([P, ns], f32, space="PSUM", tag="dps",
                                 name=f"pz{dt}") for dt in range(kd)]
                for dt in range(kd):
                    for m in range(kd):
                        nc.tensor.matmul(
                            pz[dt][:], lhsT=w_t[0][m][dt][:],
                            rhs=uT_s[s0 // cfg.dense_n][:, m, 0:ns],
                            start=(m == 0), stop=(m == kd - 1))
                if has_bpre:
                    for dt in range(kd):
                        tmp = wpool.tile([P, cfg.dense_n], f32, tag="r1")
                        nc.vector.tensor_scalar_mul(
                            tmp[:, 0:ns], c1rep_t[:, s0:s0 + ns],
                            v1w_t[:, dt:dt + 1])
                        nc.vector.tensor_tensor(
                            out=pz[dt][:], in0=pz[dt][:], in1=tmp[:, 0:ns],
                            op=mybir.AluOpType.add)

                if mode == 1:
                    zr = zpool.tile([P, kd, cfg.dense_n], f32, tag="zr")
                    for dt in range(kd):
                        nc.scalar.activation(
                            zr[:, dt, 0:ns], pz[dt][:],
                            mybir.ActivationFunctionType.Relu,
                            bias=bias_t[0][:, dt:dt + 1], scale=1.0)
                    final = zr
                else:
                    rT = zpool.tile([P, kd, cfg.dense_n], gdt, tag="zr")
                    for dt in range(kd):
                        nc.scalar.activation(
                            rT[:, dt, 0:ns], pz[dt][:],
                            mybir.ActivationFunctionType.Relu,
                            bias=bias_t[0][:, dt:dt + 1], scale=1.0)
                    py = [dpsum.tile([P, ns], f32, space="PSUM", tag="dps",
                                     name=f"py{dt}") for dt in range(kd)]
                    for dt in range(kd):
                        for m in range(kd):
                            nc.tensor.matmul(
                                py[dt][:], lhsT=w_t[1][m][dt][:],
                                rhs=rT[:, m, 0:ns],
                                start=(m == 0), stop=(m == kd - 1))
                    yT = zpool.tile([P, kd, cfg.dense_n], f32, tag="yT")
                    for dt in range(kd):
                        nc.scalar.activation(
                            yT[:, dt, 0:ns], py[dt][:],
                            mybir.ActivationFunctionType.Identity,
                            bias=bias_t[1][:, dt:dt + 1], scale=1.0)
                    final = yT

                for jj in range(ns // P):
                    blk = (s0 + jj * P) // P
                    ost = stpool.tile([P, d], f32, tag="ost")
                    for dt in range(kd):
                        ptr2 = trpsum.tile([P, P], f32, space="PSUM", tag="ptr")
                        nc.tensor.transpose(
                            out=ptr2[:], in_=final[:, dt, jj * P:(jj + 1) * P],
                            identity=ident[:])
                        nc.vector.tensor_copy(
                            ost[:, dt * P:(dt + 1) * P], ptr2[:])
                    nc.sync.dma_start(out_d[blk * P:(blk + 1) * P, :], ost[:])

    nc.compile()
    return nc


# ------------------------------------------------------------------ driver


def _run(cfg, nc_prog, per_core_common, per_core_vars, trace=False):
    in_maps = []
    for c in range(cfg.nc):
        m = dict(per_core_common)
        m.update(per_core_vars[c])
        in_maps.append(m)
    res = run_bass_kernel_spmd(nc_prog, in_maps, core_ids=list(range(cfg.nc)),
                               trace=trace)
    return res


def gcn_forward(cfg, x, edge_index, W_pre, b_pre, W1, b1, W2, b2, W_post,
                b_post, trace=False, ret_times=None):
    x = np.asarray(x, np.float32)
    src = np.asarray(edge_index[0], np.int64)
    dst = np.asarray(edge_index[1], np.int64)
    W_pre, W1, W2, W_post = (np.asarray(w, np.float32)
                             for w in (W_pre, W1, W2, W_post))
    b_pre, b1, b2, b_post = (np.asarray(b, np.float32)
                             for b in (b_pre, b1, b2, b_post))

    n, d, nl, nb, npad = cfg.n_nodes, cfg.d, cfg.nloc, cfg.nblk, cfg.npad
    deg = (np.bincount(dst, minlength=n) + 1).astype(np.float64)
    dinv = (1.0 / np.sqrt(deg)).astype(np.float32)

    loops = np.arange(n, dtype=np.int64)
    src_all = np.concatenate([src, loops])
    dst_all = np.concatenate([dst, loops])
    TA, TB, edge_planes = _prep_edges(cfg, src_all, dst_all)

    xs = x * dinv[:, None]
    WA = (W_pre.astype(np.float64) @ W1.astype(np.float64)).astype(np.float32)

    has_bpre = bool(np.any(b_pre != 0))
    dinv_cols = [
        _wrap_cols(dinv[c * nl:(c + 1) * nl], nb, npad) for c in range(cfg.nc)]

    # ---------- launch 1
    prog1 = build_launch(cfg, 1, TA, TB, has_bpre=has_bpre)
    tdt = ml_dtypes.bfloat16 if SCAT_BF16 else np.float32
    common1 = {
        "tablo": xs[: cfg.split].astype(tdt),
        "tabhi": xs[cfg.split:].astype(tdt),
        "w0": WA.astype(tdt),
        "bias0": b1.reshape(d // P, P).T.copy(),
    }
    if has_bpre:
        v1 = (b_pre.astype(np.float64) @ W1.astype(np.float64)).astype(
            np.float32)
        common1["v1w"] = v1.reshape(d // P, P).T.copy()
        # c1[dst] = (s[dst] + dinv[dst]) * dinv[dst],  s = sum_e dinv[src]
        s = np.zeros(n, np.float64)
        np.add.at(s, dst, dinv[src].astype(np.float64))
        c1_full = ((s + dinv) * dinv).astype(np.float32)
    vars1 = []
    for c in range(cfg.nc):
        v = {
            "idxp": edge_planes[c]["idxp"],
            "slotp": edge_planes[c]["slotp"],
            "dinvw": dinv_cols[c],
        }
        if has_bpre:
            cl = np.zeros(npad, np.float32)
            cl[:nl] = c1_full[c * nl:(c + 1) * nl]
            v["c1rep"] = np.tile(cl, (P, 1))
        vars1.append(v)
    res1 = _run(cfg, prog1, common1, vars1, trace=trace)
    g1 = np.concatenate([res1.results[c]["out"][:nl] for c in range(cfg.nc)])
    g1 *= dinv[:, None]
    if ret_times is not None:
        ret_times.append(res1.exec_time_ns)

    # ---------- launch 2
    prog2 = build_launch(cfg, 2, TA, TB, has_bpre=False)
    common2 = {
        "tablo": g1[: cfg.split].astype(tdt),
        "tabhi": g1[cfg.split:].astype(tdt),
        "w0": W2.astype(tdt),
        "w1": W_post.astype(tdt),
        "bias0": b2.reshape(d // P, P).T.copy(),
        "bias1": b_post.reshape(d // P, P).T.copy(),
    }
    vars2 = []
    for c in range(cfg.nc):
        vars2.append({
            "idxp": edge_planes[c]["idxp"],
            "slotp": edge_planes[c]["slotp"],
            "dinvw": dinv_cols[c],
        })
    res2 = _run(cfg, prog2, common2, vars2, trace=trace)
    y = np.concatenate([res2.results[c]["out"][:nl] for c in range(cfg.nc)])
    if ret_times is not None:
        ret_times.append(res2.exec_time_ns)
    return y


def kernel(x, edge_index, W_pre, b_pre, W1, b1, W2, b2, W_post, b_post):
    cfg = Cfg()
    return gcn_forward(cfg, x, edge_index, W_pre, b_pre, W1, b1, W2, b2,
                       W_post, b_post)


# revision 23
# speedup vs baseline: 1.0912x; 1.0027x over previous
"""GCN NodeAttributeAggregator on 8 Trainium2 NeuronCores.

Strategy (node-sharded, dst-partitioned edges):
  - Host precomputes index metadata: degrees (with self-loops), dinv=rsqrt(deg),
    per-core edge lists partitioned by dst owner, grouped by (dst-block of 128,
    src<32768 vs src>=32768 for int16 gather indices), padded to 128-edge tiles.
  - Device per core: dma_gather rows of a pre-scaled node table (xs = x*dinv),
    scatter-by-matmul: one-hot P matrices (built on DVE via iota + is_equal)
    contract each 128-edge tile into a 128-dst PSUM accumulator.
  - Dense 256x256 matmuls run in float32r (full PE rate) feature-major, with
    PE transposes at layout boundaries.
  - Algebra: GCN layer out = D^-1/2 (A+I) D^-1/2 h W.  Row scaling commutes
    with right matmuls, relu commutes with positive row scaling, and
    agg(h W) = agg(h) W, so:
      L1 (fused W_pre@W1): u' = (scatter(xs) + xs_dst) * dinv;
          g1 = relu(u' @ (W_pre W1) + b1 + rank1(b_pre)) * dinv
      L2: v' = (scatter(g1) + g1_dst) * dinv;
          y  = relu(v' @ W2 + b2) @ W_post + b_post
  - Two SPMD launches; host concatenates g1 slices between them.
"""

import dataclasses
import ml_dtypes
import numpy as np

import concourse.bacc as bacc
import concourse.bass as bass
import concourse.tile as tile
import concourse.mybir as mybir
from concourse.bass_utils import run_bass_kernel_spmd
from concourse.masks import make_identity

P = 128
SCAT_BF16 = True  # bf16 gather tables + P matrices (PE 1 cyc/row + FWL)
NSWQ = 4  # SWDGE queues
GCH = 8  # gather tiles per dma_gather call
f32 = mybir.dt.float32
f32r = mybir.dt.float32r
bf16 = mybir.dt.bfloat16
gdt = bf16 if SCAT_BF16 else f32r
i16 = mybir.dt.int16
i32 = mybir.dt.int32


@dataclasses.dataclass
class Cfg:
    n_nodes: int = 50000
    d: int = 256
    nc: int = 8
    split: int = 32768
    dense_n: int = 512

    @property
    def nloc(self):
        return self.n_nodes // self.nc

    @property
    def nblk(self):
        return (self.nloc + P - 1) // P

    @property
    def npad(self):
        return self.nblk * P

    @property
    def nhi(self):
        return self.n_nodes - self.split


# ---------------------------------------------------------------- host prep


def _wrap16(vals):
    """[n] -> [128, n//16] int16: value i at [i%16, i//16], replicated x8."""
    w = vals.reshape(-1, 16).T
    return np.tile(w, (8, 1)).astype(np.int16)


def _prep_edges(cfg, src, dst):
    """Partition edges by dst owner; group by (block, src-range); pad.

    Returns (TA, TB, per-core dict of idx/slot planes) with identical
    compile-time schedule (TA, TB) across cores.
    """
    nl, nb = cfg.nloc, cfg.nblk
    owner = dst // nl
    loc = dst - owner * nl
    blk = loc // P
    slot = loc - blk * P
    grp = (src >= cfg.split).astype(np.int64)

    key = (owner * nb + blk) * 2 + grp
    nkeys = cfg.nc * nb * 2
    counts = np.bincount(key, minlength=nkeys)
    T_cbg = -(-counts // P).reshape(cfg.nc, nb, 2)  # ceil
    T = T_cbg.max(axis=0)  # [nb, 2] tiles per (block, grp), same for all cores
    TA, TB = T[:, 0].copy(), T[:, 1].copy()
    padlen = T * P  # [nb, 2] padded edge rows

    # per-core stream layout: for b: [A rows pad][B rows pad]
    seg_len = padlen.sum(axis=1)  # per block
    blk_base = np.concatenate([[0], np.cumsum(seg_len)])[:-1]  # [nb]
    rows_total = int(seg_len.sum())
    # row base for (b, g)
    gbase = np.stack([blk_base, blk_base + padlen[:, 0]], axis=1)  # [nb, 2]

    # stable order by key, then rank within group
    order = np.argsort(key, kind="stable")
    skey = key[order]
    group_start = np.concatenate([[0], np.cumsum(np.bincount(skey, minlength=nkeys))])
    rank = np.arange(len(src)) - group_start[skey]

    rows = gbase[blk[order], grp[order]] + rank
    cores = owner[order]
    idxv = (src[order] - grp[order] * cfg.split).astype(np.int16)
    slotv = slot[order].astype(np.float32)

    idx_flat = np.zeros((cfg.nc, rows_total), np.int16)
    slot_flat = np.full((cfg.nc, rows_total), 300.0, np.float32)
    idx_flat[cores, rows] = idxv
    slot_flat[cores, rows] = slotv

    per_core = []
    for c in range(cfg.nc):
        idxp = _wrap16(idx_flat[c])  # [128, rows_total//16]
        slotp = slot_flat[c].reshape(-1, P).T.copy()  # [128, ntiles]
        per_core.append({"idxp": idxp, "slotp": slotp})
    return TA, TB, per_core


def _wrap_cols(vec, nblk, npad):
    """[npad] -> [128, nblk] with [p, b] = vec[b*128+p]."""
    v = np.zeros(npad, np.float32)
    v[: len(vec)] = vec
    return v.reshape(nblk, P).T.copy()


# ------------------------------------------------------------- device build


def build_launch(cfg, mode, TA, TB, has_bpre=False):
    """mode 1: out = relu(u' @ WA + b1 [+ rank1]) * dinv   (writes g1)
    mode 2: out = relu(v' @ W2 + b2) @ W_post + b_post     (writes y)
    """
    nb, npad, d = cfg.nblk, cfg.npad, cfg.d
    ntiles = int((TA + TB).sum())
    nidxcol = ntiles * 8
    tmaxA = max(int(TA.max()), 1)
    tmaxB = max(int(TB.max()), 1)
    tmaxP = max(int((TA + TB).max()), 1)

    nc = bacc.Bacc("TRN2", target_bir_lowering=False, debug=False,
                   num_devices=cfg.nc, num_swdge_queues=NSWQ)

    tablo = nc.dram_tensor("tablo", [cfg.split, d], gdt, kind="ExternalInput")
    tabhi = nc.dram_tensor("tabhi", [cfg.nhi, d], gdt, kind="ExternalInput")
    idxp_d = nc.dram_tensor("idxp", [P, nidxcol], i16, kind="ExternalInput")
    slotp_d = nc.dram_tensor("slotp", [P, ntiles], f32, kind="ExternalInput")
    dinvw_d = nc.dram_tensor("dinvw", [P, nb], f32, kind="ExternalInput")
    nw = 1 if mode == 1 else 2
    w_d = [nc.dram_tensor(f"w{i}", [d, d], f32r, kind="ExternalInput")
           for i in range(nw)]
    bias_d = [nc.dram_tensor(f"bias{i}", [P, d // P], f32, kind="ExternalInput")
              for i in range(nw)]
    if has_bpre:
        c1rep_d = nc.dram_tensor("c1rep", [P, npad], f32, kind="ExternalInput")
        v1w_d = nc.dram_tensor("v1w", [P, d // P], f32, kind="ExternalInput")
    out_d = nc.dram_tensor("out", [npad, d], f32, kind="ExternalOutput")

    kd = d // P  # feature k-tiles (2)

    with tile.TileContext(nc) as tc:
        with (
            tc.tile_pool(name="const", bufs=1) as cpool,
            tc.tile_pool(name="gA", bufs=6) as gApool,
            tc.tile_pool(name="gB", bufs=6) as gBpool,
            tc.tile_pool(name="pmat", bufs=2) as ppool,
            tc.tile_pool(name="work", bufs=3) as wpool,
            tc.tile_pool(name="stage", bufs=3) as stpool,
            tc.tile_pool(name="zslab", bufs=2) as zpool,
            tc.tile_pool(name="apsum", bufs=4, space="PSUM") as apsum,
            tc.tile_pool(name="trpsum", bufs=2, space="PSUM") as trpsum,
            tc.tile_pool(name="dpsum", bufs=2, space="PSUM") as dpsum,
        ):
            # ---- constants
            iota_i = cpool.tile([P, P], i32)
            nc.gpsimd.iota(iota_i[:], pattern=[[1, P]], base=0,
                           channel_multiplier=0)
            iota_f = cpool.tile([P, P], f32)
            nc.vector.tensor_copy(iota_f[:], iota_i[:])
            ident = cpool.tile([P, P], f32)
            make_identity(nc, ident[:])
            idxp_t = cpool.tile([P, nidxcol], i16)
            nc.sync.dma_start(idxp_t[:], idxp_d[:])
            slotp_t = cpool.tile([P, ntiles], f32)
            nc.sync.dma_start(slotp_t[:], slotp_d[:])
            dinvw_t = cpool.tile([P, nb], f32)
            nc.sync.dma_start(dinvw_t[:], dinvw_d[:])
            w_t = []  # [stage][k][m] -> [128,128] f32r lhsT tiles
            for i in range(nw):
                tiles = []
                for k in range(kd):
                    row = []
                    for m in range(kd):
                        wt = cpool.tile([P, P], f32r, name=f"wt{i}_{k}_{m}",
                                        tag=f"wt{i}_{k}_{m}")
                        nc.sync.dma_start(
                            wt[:], w_d[i][k * P:(k + 1) * P, m * P:(m + 1) * P])
                        row.append(wt)
                    tiles.append(row)
                w_t.append(tiles)
            bias_t = []
            for i in range(nw):
                bt = cpool.tile([P, kd], f32, name=f"bt{i}", tag=f"bt{i}")
                nc.sync.dma_start(bt[:], bias_d[i][:])
                bias_t.append(bt)
            if has_bpre:
                c1rep_t = cpool.tile([P, npad], f32)
                nc.sync.dma_start(c1rep_t[:], c1rep_d[:])
                v1w_t = cpool.tile([P, kd], f32)
                nc.sync.dma_start(v1w_t[:], v1w_d[:])

            # feature-major activations, one tile per dense node-slice
            nsl = (npad + cfg.dense_n - 1) // cfg.dense_n
            uT_s = [cpool.tile([P, kd, min(cfg.dense_n, npad - i * cfg.dense_n)],
                               f32r, name=f"uTs{i}", tag=f"uTs{i}")
                    for i in range(nsl)]

            # ---- aggregation pass
            gq = [0]
            icol = 0  # idx plane column cursor (units of 8 per tile)
            tcol = 0  # slot plane column cursor (1 per tile)
            for b in range(nb):
                ta, tb = int(TA[b]), int(TB[b])
                tbt = ta + tb
                assert tbt > 0
                psum_a = apsum.tile([P, d], f32, space="PSUM", tag="psum_a")
                if tbt:
                    p_all = ppool.tile([P, tmaxP, P], gdt, tag="pmat")
                    nc.vector.tensor_tensor(
                        out=p_all[:, 0:tbt, :],
                        in0=slotp_t[:, tcol:tcol + tbt, None].to_broadcast(
                            [P, tbt, P]),
                        in1=iota_f[:, None, :].to_broadcast([P, tbt, P]),
                        op=mybir.AluOpType.is_equal)
                j = 0
                for pool_g, tab_ap, tcnt, gtag in (
                    (gApool, tablo, ta, "gA"),
                    (gBpool, tabhi, tb, "gB"),
                ):
                    for c0 in range(0, tcnt, GCH):
                        cn = min(GCH, tcnt - c0)
                        gt = pool_g.tile([P, GCH, d], gdt, tag=gtag,
                                         name=f"g_{b}_{gtag}_{c0}")
                        nc.gpsimd.dma_gather(
                            out_ap=gt[:, 0:cn, :], in_ap=tab_ap[:],
                            idxs_ap=idxp_t[:, icol:icol + cn * 8],
                            num_idxs=cn * P, num_idxs_reg=cn * P, elem_size=d,
                            queue_num=gq[0] % NSWQ)
                        gq[0] += 1
                        icol += cn * 8
                        for t in range(cn):
                            nc.tensor.matmul(
                                psum_a[:], lhsT=p_all[:, j, :], rhs=gt[:, t, :],
                                start=(j == 0), stop=(j == tbt - 1))
                            j += 1
                tcol += tbt

                # epilogue: u' = psum * dinv (self-loops are gathered edges)
                u2 = wpool.tile([P, d], f32, tag="u2")
                nc.scalar.mul(u2[:], psum_a[:], dinvw_t[:, b:b + 1])
                for m in range(kd):
                    ptr = trpsum.tile([P, P], f32, space="PSUM", tag="ptr")
                    nc.tensor.transpose(out=ptr[:], in_=u2[:, m * P:(m + 1) * P],
                                        identity=ident[:])
                    sl, off = divmod(b * P, cfg.dense_n)
                    nc.vector.tensor_copy(uT_s[sl][:, m, off:off + P], ptr[:])

            # ---- dense pass over node slices
            for s0 in range(0, npad, cfg.dense_n):
                ns = min(cfg.dense_n, npad - s0)
                pz = [dpsum.tile([P, ns], f32, space="PSUM", tag="dps",
                                 name=f"pz{dt}") for dt in range(kd)]
                for dt in range(kd):
                    for m in range(kd):
                        nc.tensor.matmul(
                            pz[dt][:], lhsT=w_t[0][m][dt][:],
                            rhs=uT_s[s0 // cfg.dense_n][:, m, 0:ns],
                            start=(m == 0), stop=(m == kd - 1))
                if has_bpre:
                    for dt in range(kd):
                        tmp = wpool.tile([P, cfg.dense_n], f32, tag="r1")
                        nc.vector.tensor_scalar_mul(
                            tmp[:, 0:ns], c1rep_t[:, s0:s0 + ns],
                            v1w_t[:, dt:dt + 1])
                        nc.vector.tensor_tensor(
                            out=pz[dt][:], in0=pz[dt][:], in1=tmp[:, 0:ns],
                            op=mybir.AluOpType.add)

                if mode == 1:
                    zr = zpool.tile([P, kd, cfg.dense_n], f32, tag="zr")
                    for dt in range(kd):
                        nc.scalar.activation(
                            zr[:, dt, 0:ns], pz[dt][:],
                            mybir.ActivationFunctionType.Relu,
                            bias=bias_t[0][:, dt:dt + 1], scale=1.0)
                    final = zr
                else:
                    rT = zpool.tile([P, kd, cfg.dense_n], f32r, tag="zr")
                    for dt in range(kd):
                        nc.scalar.activation(
                            rT[:, dt, 0:ns], pz[dt][:],
                            mybir.ActivationFunctionType.Relu,
                            bias=bias_t[0][:, dt:dt + 1], scale=1.0)
                    py = [dpsum.tile([P, ns], f32, space="PSUM", tag="dps",
                                     name=f"py{dt}") for dt in range(kd)]
                    for dt in range(kd):
                        for m in range(kd):
                            nc.tensor.matmul(
                                py[dt][:], lhsT=w_t[1][m][dt][:],
                                rhs=rT[:, m, 0:ns],
                                start=(m == 0), stop=(m == kd - 1))
                    yT = zpool.tile([P, kd, cfg.dense_n], f32, tag="yT")
                    for dt in range(kd):
                        nc.scalar.activation(
                            yT[:, dt, 0:ns], py[dt][:],
                            mybir.ActivationFunctionType.Identity,
                            bias=bias_t[1][:, dt:dt + 1], scale=1.0)
                    final = yT

                for jj in range(ns // P):
                    blk = (s0 + jj * P) // P
                    ost = stpool.tile([P, d], f32, tag="ost")
                    for dt in range(kd):
                        ptr2 = trpsum.tile([P, P], f32, space="PSUM", tag="ptr")
                        nc.tensor.transpose(
                            out=ptr2[:], in_=final[:, dt, jj * P:(jj + 1) * P],
                            identity=ident[:])
                        nc.vector.tensor_copy(
                            ost[:, dt * P:(dt + 1) * P], ptr2[:])
                    nc.sync.dma_start(out_d[blk * P:(blk + 1) * P, :], ost[:])

    nc.compile()
    return nc


# ------------------------------------------------------------------ driver


def _run(cfg, nc_prog, per_core_common, per_core_vars, trace=False):
    in_maps = []
    for c in range(cfg.nc):
        m = dict(per_core_common)
        m.update(per_core_vars[c])
        in_maps.append(m)
    res = run_bass_kernel_spmd(nc_prog, in_maps, core_ids=list(range(cfg.nc)),
                               trace=trace)
    return res


def gcn_forward(cfg, x, edge_index, W_pre, b_pre, W1, b1, W2, b2, W_post,
                b_post, trace=False, ret_times=None):
    x = np.asarray(x, np.float32)
    src = np.asarray(edge_index[0], np.int64)
    dst = np.asarray(edge_index[1], np.int64)
    W_pre, W1, W2, W_post = (np.asarray(w, np.float32)
                             for w in (W_pre, W1, W2, W_post))
    b_pre, b1, b2, b_post = (np.asarray(b, np.float32)
                             for b in (b_pre, b1, b2, b_post))

    n, d, nl, nb, npad = cfg.n_nodes, cfg.d, cfg.nloc, cfg.nblk, cfg.npad
    deg = (np.bincount(dst, minlength=n) + 1).astype(np.float64)
    dinv = (1.0 / np.sqrt(deg)).astype(np.float32)

    loops = np.arange(n, dtype=np.int64)
    src_all = np.concatenate([src, loops])
    dst_all = np.concatenate([dst, loops])
    TA, TB, edge_planes = _prep_edges(cfg, src_all, dst_all)

    xs = x * dinv[:, None]
    WA = (W_pre.astype(np.float64) @ W1.astype(np.float64)).astype(np.float32)

    has_bpre = bool(np.any(b_pre != 0))
    dinv_cols = [
        _wrap_cols(dinv[c * nl:(c + 1) * nl], nb, npad) for c in range(cfg.nc)]

    # ---------- launch 1
    prog1 = build_launch(cfg, 1, TA, TB, has_bpre=has_bpre)
    tdt = ml_dtypes.bfloat16 if SCAT_BF16 else np.float32
    common1 = {
        "tablo": xs[: cfg.split].astype(tdt),
        "tabhi": xs[cfg.split:].astype(tdt),
        "w0": WA,
        "bias0": b1.reshape(d // P, P).T.copy(),
    }
    if has_bpre:
        v1 = (b_pre.astype(np.float64) @ W1.astype(np.float64)).astype(
            np.float32)
        common1["v1w"] = v1.reshape(d // P, P).T.copy()
        # c1[dst] = (s[dst] + dinv[dst]) * dinv[dst],  s = sum_e dinv[src]
        s = np.zeros(n, np.float64)
        np.add.at(s, dst, dinv[src].astype(np.float64))
        c1_full = ((s + dinv) * dinv).astype(np.float32)
    vars1 = []
    for c in range(cfg.nc):
        v = {
            "idxp": edge_planes[c]["idxp"],
            "slotp": edge_planes[c]["slotp"],
            "dinvw": dinv_cols[c],
        }
        if has_bpre:
            cl = np.zeros(npad, np.float32)
            cl[:nl] = c1_full[c * nl:(c + 1) * nl]
            v["c1rep"] = np.tile(cl, (P, 1))
        vars1.append(v)
    res1 = _run(cfg, prog1, common1, vars1, trace=trace)
    g1 = np.concatenate([res1.results[c]["out"][:nl] for c in range(cfg.nc)])
    g1 *= dinv[:, None]
    if ret_times is not None:
        ret_times.append(res1.exec_time_ns)

    # ---------- launch 2
    prog2 = build_launch(cfg, 2, TA, TB, has_bpre=False)
    common2 = {
        "tablo": g1[: cfg.split].astype(tdt),
        "tabhi": g1[cfg.split:].astype(tdt),
        "w0": W2,
        "w1": W_post,
        "bias0": b2.reshape(d // P, P).T.copy(),
        "bias1": b_post.reshape(d // P, P).T.copy(),
    }
    vars2 = []
    for c in range(cfg.nc):
        vars2.append({
            "idxp": edge_planes[c]["idxp"],
            "slotp": edge_planes[c]["slotp"],
            "dinvw": dinv_cols[c],
        })
    res2 = _run(cfg, prog2, common2, vars2, trace=trace)
    y = np.concatenate([res2.results[c]["out"][:nl] for c in range(cfg.nc)])
    if ret_times is not None:
        ret_times.append(res2.exec_time_ns)
    return y


def kernel(x, edge_index, W_pre, b_pre, W1, b1, W2, b2, W_post, b_post):
    cfg = Cfg()
    return gcn_forward(cfg, x, edge_index, W_pre, b_pre, W1, b1, W2, b2,
                       W_post, b_post)


# revision 24
# speedup vs baseline: 1.1930x; 1.0933x over previous
"""GCN NodeAttributeAggregator on 8 Trainium2 NeuronCores.

Strategy (node-sharded, dst-partitioned edges):
  - Host precomputes index metadata: degrees (with self-loops), dinv=rsqrt(deg),
    per-core edge lists partitioned by dst owner, grouped by (dst-block of 128,
    src<32768 vs src>=32768 for int16 gather indices), padded to 128-edge tiles.
  - Device per core: dma_gather rows of a pre-scaled node table (xs = x*dinv),
    scatter-by-matmul: one-hot P matrices (built on DVE via iota + is_equal)
    contract each 128-edge tile into a 128-dst PSUM accumulator.
  - Dense 256x256 matmuls run in float32r (full PE rate) feature-major, with
    PE transposes at layout boundaries.
  - Algebra: GCN layer out = D^-1/2 (A+I) D^-1/2 h W.  Row scaling commutes
    with right matmuls, relu commutes with positive row scaling, and
    agg(h W) = agg(h) W, so:
      L1 (fused W_pre@W1): u' = (scatter(xs) + xs_dst) * dinv;
          g1 = relu(u' @ (W_pre W1) + b1 + rank1(b_pre)) * dinv
      L2: v' = (scatter(g1) + g1_dst) * dinv;
          y  = relu(v' @ W2 + b2) @ W_post + b_post
  - Two SPMD launches; host concatenates g1 slices between them.
"""

import dataclasses
import ml_dtypes
import numpy as np

import concourse.bacc as bacc
import concourse.bass as bass
import concourse.tile as tile
import concourse.mybir as mybir
from concourse.bass_utils import run_bass_kernel_spmd
from concourse.masks import make_identity

P = 128
SCAT_BF16 = True  # bf16 gather tables + P matrices (PE 1 cyc/row + FWL)
NSWQ = 4  # SWDGE queues
GCH = 8  # gather tiles per dma_gather call
f32 = mybir.dt.float32
f32r = mybir.dt.float32r
bf16 = mybir.dt.bfloat16
gdt = bf16 if SCAT_BF16 else f32r
i16 = mybir.dt.int16
i32 = mybir.dt.int32


@dataclasses.dataclass
class Cfg:
    n_nodes: int = 50000
    d: int = 256
    nc: int = 8
    split: int = 32768
    dense_n: int = 512

    @property
    def nloc(self):
        return self.n_nodes // self.nc

    @property
    def nblk(self):
        return (self.nloc + P - 1) // P

    @property
    def npad(self):
        return self.nblk * P

    @property
    def nhi(self):
        return self.n_nodes - self.split


# ---------------------------------------------------------------- host prep


def _wrap16(vals):
    """[n] -> [128, n//16] int16: value i at [i%16, i//16], replicated x8."""
    w = vals.reshape(-1, 16).T
    return np.tile(w, (8, 1)).astype(np.int16)


def _prep_edges(cfg, src, dst):
    """Partition edges by dst owner; group by (block, src-range); pad.

    Returns (TA, TB, per-core dict of idx/slot planes) with identical
    compile-time schedule (TA, TB) across cores.
    """
    nl, nb = cfg.nloc, cfg.nblk
    owner = dst // nl
    loc = dst - owner * nl
    blk = loc // P
    slot = loc - blk * P
    grp = (src >= cfg.split).astype(np.int64)

    key = (owner * nb + blk) * 2 + grp
    nkeys = cfg.nc * nb * 2
    counts = np.bincount(key, minlength=nkeys)
    T_cbg = -(-counts // P).reshape(cfg.nc, nb, 2)  # ceil
    T = T_cbg.max(axis=0)  # [nb, 2] tiles per (block, grp), same for all cores
    TA, TB = T[:, 0].copy(), T[:, 1].copy()
    padlen = T * P  # [nb, 2] padded edge rows

    # per-core stream layout: for b: [A rows pad][B rows pad]
    seg_len = padlen.sum(axis=1)  # per block
    blk_base = np.concatenate([[0], np.cumsum(seg_len)])[:-1]  # [nb]
    rows_total = int(seg_len.sum())
    # row base for (b, g)
    gbase = np.stack([blk_base, blk_base + padlen[:, 0]], axis=1)  # [nb, 2]

    # stable order by key, then rank within group
    order = np.argsort(key, kind="stable")
    skey = key[order]
    group_start = np.concatenate([[0], np.cumsum(np.bincount(skey, minlength=nkeys))])
    rank = np.arange(len(src)) - group_start[skey]

    rows = gbase[blk[order], grp[order]] + rank
    cores = owner[order]
    idxv = (src[order] - grp[order] * cfg.split).astype(np.int16)
    slotv = slot[order].astype(np.float32)

    idx_flat = np.zeros((cfg.nc, rows_total), np.int16)
    slot_flat = np.full((cfg.nc, rows_total), 300.0, np.float32)
    idx_flat[cores, rows] = idxv
    slot_flat[cores, rows] = slotv

    per_core = []
    for c in range(cfg.nc):
        idxp = _wrap16(idx_flat[c])  # [128, rows_total//16]
        slotp = slot_flat[c].reshape(-1, P).T.copy()  # [128, ntiles]
        per_core.append({"idxp": idxp, "slotp": slotp})
    return TA, TB, per_core


def _wrap_cols(vec, nblk, npad):
    """[npad] -> [128, nblk] with [p, b] = vec[b*128+p]."""
    v = np.zeros(npad, np.float32)
    v[: len(vec)] = vec
    return v.reshape(nblk, P).T.copy()


# ------------------------------------------------------------- device build


def build_launch(cfg, mode, TA, TB, has_bpre=False):
    """mode 1: out = relu(u' @ WA + b1 [+ rank1]) * dinv   (writes g1)
    mode 2: out = relu(v' @ W2 + b2) @ W_post + b_post     (writes y)
    """
    nb, npad, d = cfg.nblk, cfg.npad, cfg.d
    ntiles = int((TA + TB).sum())
    nidxcol = ntiles * 8
    tmaxA = max(int(TA.max()), 1)
    tmaxB = max(int(TB.max()), 1)
    tmaxP = max(int((TA + TB).max()), 1)

    nc = bacc.Bacc("TRN2", target_bir_lowering=False, debug=False,
                   num_devices=cfg.nc, num_swdge_queues=NSWQ)

    tablo = nc.dram_tensor("tablo", [cfg.split, d], gdt, kind="ExternalInput")
    loctab = nc.dram_tensor("loctab", [npad, d], gdt, kind="ExternalInput")
    tabhi = nc.dram_tensor("tabhi", [cfg.nhi, d], gdt, kind="ExternalInput")
    idxp_d = nc.dram_tensor("idxp", [P, nidxcol], i16, kind="ExternalInput")
    slotp_d = nc.dram_tensor("slotp", [P, ntiles], f32, kind="ExternalInput")
    dinvw_d = nc.dram_tensor("dinvw", [P, nb], f32, kind="ExternalInput")
    nw = 1 if mode == 1 else 2
    w_d = [nc.dram_tensor(f"w{i}", [d, d], f32r, kind="ExternalInput")
           for i in range(nw)]
    bias_d = [nc.dram_tensor(f"bias{i}", [P, d // P], f32, kind="ExternalInput")
              for i in range(nw)]
    if has_bpre:
        c1rep_d = nc.dram_tensor("c1rep", [P, npad], f32, kind="ExternalInput")
        v1w_d = nc.dram_tensor("v1w", [P, d // P], f32, kind="ExternalInput")
    out_d = nc.dram_tensor("out", [npad, d], f32, kind="ExternalOutput")

    kd = d // P  # feature k-tiles (2)

    with tile.TileContext(nc) as tc:
        with (
            tc.tile_pool(name="const", bufs=1) as cpool,
            tc.tile_pool(name="gA", bufs=6) as gApool,
            tc.tile_pool(name="gB", bufs=6) as gBpool,
            tc.tile_pool(name="loc", bufs=4) as locpool,
            tc.tile_pool(name="pmat", bufs=2) as ppool,
            tc.tile_pool(name="work", bufs=3) as wpool,
            tc.tile_pool(name="stage", bufs=3) as stpool,
            tc.tile_pool(name="zslab", bufs=2) as zpool,
            tc.tile_pool(name="apsum", bufs=4, space="PSUM") as apsum,
            tc.tile_pool(name="trpsum", bufs=2, space="PSUM") as trpsum,
            tc.tile_pool(name="dpsum", bufs=2, space="PSUM") as dpsum,
        ):
            # ---- constants
            iota_i = cpool.tile([P, P], i32)
            nc.gpsimd.iota(iota_i[:], pattern=[[1, P]], base=0,
                           channel_multiplier=0)
            iota_f = cpool.tile([P, P], f32)
            nc.vector.tensor_copy(iota_f[:], iota_i[:])
            ident = cpool.tile([P, P], f32)
            make_identity(nc, ident[:])
            ident_g = cpool.tile([P, P], gdt)
            nc.vector.tensor_copy(ident_g[:], ident[:])
            idxp_t = cpool.tile([P, nidxcol], i16)
            nc.sync.dma_start(idxp_t[:], idxp_d[:])
            slotp_t = cpool.tile([P, ntiles], f32)
            nc.sync.dma_start(slotp_t[:], slotp_d[:])
            dinvw_t = cpool.tile([P, nb], f32)
            nc.sync.dma_start(dinvw_t[:], dinvw_d[:])
            w_t = []  # [stage][k][m] -> [128,128] f32r lhsT tiles
            for i in range(nw):
                tiles = []
                for k in range(kd):
                    row = []
                    for m in range(kd):
                        wt = cpool.tile([P, P], f32r, name=f"wt{i}_{k}_{m}",
                                        tag=f"wt{i}_{k}_{m}")
                        nc.sync.dma_start(
                            wt[:], w_d[i][k * P:(k + 1) * P, m * P:(m + 1) * P])
                        row.append(wt)
                    tiles.append(row)
                w_t.append(tiles)
            bias_t = []
            for i in range(nw):
                bt = cpool.tile([P, kd], f32, name=f"bt{i}", tag=f"bt{i}")
                nc.sync.dma_start(bt[:], bias_d[i][:])
                bias_t.append(bt)
            if has_bpre:
                c1rep_t = cpool.tile([P, npad], f32)
                nc.sync.dma_start(c1rep_t[:], c1rep_d[:])
                v1w_t = cpool.tile([P, kd], f32)
                nc.sync.dma_start(v1w_t[:], v1w_d[:])

            # feature-major activations, one tile per dense node-slice
            nsl = (npad + cfg.dense_n - 1) // cfg.dense_n
            uT_s = [cpool.tile([P, kd, min(cfg.dense_n, npad - i * cfg.dense_n)],
                               f32r, name=f"uTs{i}", tag=f"uTs{i}")
                    for i in range(nsl)]

            # ---- aggregation pass
            gq = [0]
            icol = 0  # idx plane column cursor (units of 8 per tile)
            tcol = 0  # slot plane column cursor (1 per tile)
            for b in range(nb):
                ta, tb = int(TA[b]), int(TB[b])
                tbt = ta + tb
                assert tbt > 0
                psum_a = apsum.tile([P, d], f32, space="PSUM", tag="psum_a")
                if tbt:
                    p_all = ppool.tile([P, tmaxP, P], gdt, tag="pmat")
                    nc.vector.tensor_tensor(
                        out=p_all[:, 0:tbt, :],
                        in0=slotp_t[:, tcol:tcol + tbt, None].to_broadcast(
                            [P, tbt, P]),
                        in1=iota_f[:, None, :].to_broadcast([P, tbt, P]),
                        op=mybir.AluOpType.is_equal)
                selft = locpool.tile([P, d], gdt, tag="selft")
                nc.sync.dma_start(selft[:], loctab[b * P:(b + 1) * P, :])
                nc.tensor.matmul(psum_a[:], lhsT=ident_g[:], rhs=selft[:],
                                 start=True, stop=False)
                j = 0
                for pool_g, tab_ap, tcnt, gtag in (
                    (gApool, tablo, ta, "gA"),
                    (gBpool, tabhi, tb, "gB"),
                ):
                    for c0 in range(0, tcnt, GCH):
                        cn = min(GCH, tcnt - c0)
                        gt = pool_g.tile([P, GCH, d], gdt, tag=gtag,
                                         name=f"g_{b}_{gtag}_{c0}")
                        nc.gpsimd.dma_gather(
                            out_ap=gt[:, 0:cn, :], in_ap=tab_ap[:],
                            idxs_ap=idxp_t[:, icol:icol + cn * 8],
                            num_idxs=cn * P, num_idxs_reg=cn * P, elem_size=d,
                            queue_num=gq[0] % NSWQ)
                        gq[0] += 1
                        icol += cn * 8
                        for t in range(cn):
                            nc.tensor.matmul(
                                psum_a[:], lhsT=p_all[:, j, :], rhs=gt[:, t, :],
                                start=False, stop=(j == tbt - 1))
                            j += 1
                tcol += tbt

                # epilogue: u' = psum * dinv (self-loops are gathered edges)
                u2 = wpool.tile([P, d], f32, tag="u2")
                nc.scalar.mul(u2[:], psum_a[:], dinvw_t[:, b:b + 1])
                for m in range(kd):
                    ptr = trpsum.tile([P, P], f32, space="PSUM", tag="ptr")
                    nc.tensor.transpose(out=ptr[:], in_=u2[:, m * P:(m + 1) * P],
                                        identity=ident[:])
                    sl, off = divmod(b * P, cfg.dense_n)
                    nc.vector.tensor_copy(uT_s[sl][:, m, off:off + P], ptr[:])

            # ---- dense pass over node slices
            for s0 in range(0, npad, cfg.dense_n):
                ns = min(cfg.dense_n, npad - s0)
                pz = [dpsum.tile([P, ns], f32, space="PSUM", tag="dps",
                                 name=f"pz{dt}") for dt in range(kd)]
                for dt in range(kd):
                    for m in range(kd):
                        nc.tensor.matmul(
                            pz[dt][:], lhsT=w_t[0][m][dt][:],
                            rhs=uT_s[s0 // cfg.dense_n][:, m, 0:ns],
                            start=(m == 0), stop=(m == kd - 1))
                if has_bpre:
                    for dt in range(kd):
                        tmp = wpool.tile([P, cfg.dense_n], f32, tag="r1")
                        nc.vector.tensor_scalar_mul(
                            tmp[:, 0:ns], c1rep_t[:, s0:s0 + ns],
                            v1w_t[:, dt:dt + 1])
                        nc.vector.tensor_tensor(
                            out=pz[dt][:], in0=pz[dt][:], in1=tmp[:, 0:ns],
                            op=mybir.AluOpType.add)

                if mode == 1:
                    zr = zpool.tile([P, kd, cfg.dense_n], f32, tag="zr")
                    for dt in range(kd):
                        nc.scalar.activation(
                            zr[:, dt, 0:ns], pz[dt][:],
                            mybir.ActivationFunctionType.Relu,
                            bias=bias_t[0][:, dt:dt + 1], scale=1.0)
                    final = zr
                else:
                    rT = zpool.tile([P, kd, cfg.dense_n], f32r, tag="zr")
                    for dt in range(kd):
                        nc.scalar.activation(
                            rT[:, dt, 0:ns], pz[dt][:],
                            mybir.ActivationFunctionType.Relu,
                            bias=bias_t[0][:, dt:dt + 1], scale=1.0)
                    py = [dpsum.tile([P, ns], f32, space="PSUM", tag="dps",
                                     name=f"py{dt}") for dt in range(kd)]
                    for dt in range(kd):
                        for m in range(kd):
                            nc.tensor.matmul(
                                py[dt][:], lhsT=w_t[1][m][dt][:],
                                rhs=rT[:, m, 0:ns],
                                start=(m == 0), stop=(m == kd - 1))
                    yT = zpool.tile([P, kd, cfg.dense_n], f32, tag="yT")
                    for dt in range(kd):
                        nc.scalar.activation(
                            yT[:, dt, 0:ns], py[dt][:],
                            mybir.ActivationFunctionType.Identity,
                            bias=bias_t[1][:, dt:dt + 1], scale=1.0)
                    final = yT

                for jj in range(ns // P):
                    blk = (s0 + jj * P) // P
                    ost = stpool.tile([P, d], f32, tag="ost")
                    for dt in range(kd):
                        ptr2 = trpsum.tile([P, P], f32, space="PSUM", tag="ptr")
                        nc.tensor.transpose(
                            out=ptr2[:], in_=final[:, dt, jj * P:(jj + 1) * P],
                            identity=ident[:])
                        nc.vector.tensor_copy(
                            ost[:, dt * P:(dt + 1) * P], ptr2[:])
                    nc.sync.dma_start(out_d[blk * P:(blk + 1) * P, :], ost[:])

    nc.compile()
    return nc


# ------------------------------------------------------------------ driver


def _run(cfg, nc_prog, per_core_common, per_core_vars, trace=False):
    in_maps = []
    for c in range(cfg.nc):
        m = dict(per_core_common)
        m.update(per_core_vars[c])
        in_maps.append(m)
    res = run_bass_kernel_spmd(nc_prog, in_maps, core_ids=list(range(cfg.nc)),
                               trace=trace)
    return res


def gcn_forward(cfg, x, edge_index, W_pre, b_pre, W1, b1, W2, b2, W_post,
                b_post, trace=False, ret_times=None):
    x = np.asarray(x, np.float32)
    src = np.asarray(edge_index[0], np.int64)
    dst = np.asarray(edge_index[1], np.int64)
    W_pre, W1, W2, W_post = (np.asarray(w, np.float32)
                             for w in (W_pre, W1, W2, W_post))
    b_pre, b1, b2, b_post = (np.asarray(b, np.float32)
                             for b in (b_pre, b1, b2, b_post))

    n, d, nl, nb, npad = cfg.n_nodes, cfg.d, cfg.nloc, cfg.nblk, cfg.npad
    deg = (np.bincount(dst, minlength=n) + 1).astype(np.float64)
    dinv = (1.0 / np.sqrt(deg)).astype(np.float32)

    TA, TB, edge_planes = _prep_edges(cfg, src, dst)

    def local_pad(tab, c):
        out = np.zeros((npad, d), tab.dtype)
        out[:nl] = tab[c * nl:(c + 1) * nl]
        return out

    xs = x * dinv[:, None]
    WA = (W_pre.astype(np.float64) @ W1.astype(np.float64)).astype(np.float32)

    has_bpre = bool(np.any(b_pre != 0))
    dinv_cols = [
        _wrap_cols(dinv[c * nl:(c + 1) * nl], nb, npad) for c in range(cfg.nc)]

    # ---------- launch 1
    prog1 = build_launch(cfg, 1, TA, TB, has_bpre=has_bpre)
    tdt = ml_dtypes.bfloat16 if SCAT_BF16 else np.float32
    common1 = {
        "tablo": xs[: cfg.split].astype(tdt),
        "tabhi": xs[cfg.split:].astype(tdt),
        "w0": WA,
        "bias0": b1.reshape(d // P, P).T.copy(),
    }
    if has_bpre:
        v1 = (b_pre.astype(np.float64) @ W1.astype(np.float64)).astype(
            np.float32)
        common1["v1w"] = v1.reshape(d // P, P).T.copy()
        # c1[dst] = (s[dst] + dinv[dst]) * dinv[dst],  s = sum_e dinv[src]
        s = np.zeros(n, np.float64)
        np.add.at(s, dst, dinv[src].astype(np.float64))
        c1_full = ((s + dinv) * dinv).astype(np.float32)
    vars1 = []
    for c in range(cfg.nc):
        v = {
            "loctab": local_pad(xs.astype(tdt), c),
            "idxp": edge_planes[c]["idxp"],
            "slotp": edge_planes[c]["slotp"],
            "dinvw": dinv_cols[c],
        }
        if has_bpre:
            cl = np.zeros(npad, np.float32)
            cl[:nl] = c1_full[c * nl:(c + 1) * nl]
            v["c1rep"] = np.tile(cl, (P, 1))
        vars1.append(v)
    res1 = _run(cfg, prog1, common1, vars1, trace=trace)
    g1 = np.concatenate([res1.results[c]["out"][:nl] for c in range(cfg.nc)])
    g1 *= dinv[:, None]
    if ret_times is not None:
        ret_times.append(res1.exec_time_ns)

    # ---------- launch 2
    prog2 = build_launch(cfg, 2, TA, TB, has_bpre=False)
    common2 = {
        "tablo": g1[: cfg.split].astype(tdt),
        "tabhi": g1[cfg.split:].astype(tdt),
        "w0": W2,
        "w1": W_post,
        "bias0": b2.reshape(d // P, P).T.copy(),
        "bias1": b_post.reshape(d // P, P).T.copy(),
    }
    vars2 = []
    for c in range(cfg.nc):
        vars2.append({
            "loctab": local_pad(g1.astype(tdt), c),
            "idxp": edge_planes[c]["idxp"],
            "slotp": edge_planes[c]["slotp"],
            "dinvw": dinv_cols[c],
        })
    res2 = _run(cfg, prog2, common2, vars2, trace=trace)
    y = np.concatenate([res2.results[c]["out"][:nl] for c in range(cfg.nc)])
    if ret_times is not None:
        ret_times.append(res2.exec_time_ns)
    return y


def kernel(x, edge_index, W_pre, b_pre, W1, b1, W2, b2, W_post, b_post):
    cfg = Cfg()
    return gcn_forward(cfg, x, edge_index, W_pre, b_pre, W1, b1, W2, b2,
                       W_post, b_post)
